# revision 1
# baseline (speedup 1.0000x reference)
"""Trainium2 Bass kernel for nn_Attention_34033320854122.

Dense transformer attention block: QKV proj -> causal depthwise conv+SiLU ->
per-head RMSNorm -> partial RoPE -> causal attention -> output projection.

Sharding: tensor-parallel over the 16 heads across 8 NeuronCores (2 heads =
256 channels per core). Each core computes q/k/v for its channels (full
contraction over D), runs attention for its 2 heads, and produces a partial
output projection (outT_partial = Wo[:, cols] @ attn_cols^T). The host sums
the 8 partials and transposes.

Notes on fidelity to the reference:
- The reference negates the rotated RoPE sub-dim of BOTH q and k
  (return concat([-x_rot, x_pass])). The negation cancels exactly in
  q . k, so it is skipped.
- softmax is computed without max-subtraction: scores are O(1)-bounded
  (RMS-normed q/k, scale 1/sqrt(128)), far from fp32 exp overflow.

Matmuls run in float32r (PE's reduced-precision fp32 mode, ~13-bit
mantissa, full throughput at moving-dim >= 256). Raw fp32 bytes DMA'd into
f32r tiles behave bit-identically to the gpsimd cast-DMA path (verified).
"""

import ml_dtypes
import numpy as np

import concourse.bacc as bacc
import concourse.tile as tile
import concourse.mybir as mybir
from concourse import bass_utils
from concourse.masks import make_identity

# Problem shape (hardcoded per contract)
B, T, D = 1, 2048, 2048
H, HD = 16, 128
RD = 64
KCONV = 4
EPS = 1e-5
NCORES = 8
CPC = D // NCORES      # channels per core = 256
MPC = CPC // HD        # head tiles per core = 2
NT = 512               # free-dim tile for matmuls
NQ = T // NT           # 4 q tiles
NKC = T // HD          # 16 key chunks of 128
KD = D // 128          # 16 contraction chunks
PAD = KCONV - 1        # causal conv history

F32 = mybir.dt.float32
F32R = mybir.dt.float32r
BF16 = mybir.dt.bfloat16

_COMPILED = None


def _build():
    nc = bacc.Bacc("TRN2", target_bir_lowering=False, debug=False,
                   num_devices=NCORES)

    d = {}
    d["xT"] = nc.dram_tensor("xT", (D, T), BF16, kind="ExternalInput").ap()
    d["wqT"] = nc.dram_tensor("wqT", (D, CPC), BF16, kind="ExternalInput").ap()
    d["wkT"] = nc.dram_tensor("wkT", (D, CPC), BF16, kind="ExternalInput").ap()
    d["wvT"] = nc.dram_tensor("wvT", (D, CPC), BF16, kind="ExternalInput").ap()
    d["woT"] = nc.dram_tensor("woT", (CPC, D), F32R, kind="ExternalInput").ap()
    # trig: rows 0:64 = cos^T, rows 64:128 = sign-folded sin^T
    d["trig"] = nc.dram_tensor("trig", (128, T), F32, kind="ExternalInput").ap()
    # conv weights packed [128, proj(3), m(2), tap(4)]
    d["convw"] = nc.dram_tensor("convw", (128, 3, 2, KCONV), F32,
                                kind="ExternalInput").ap()
    # per-head norm weights [128, 2] (q, k)
    d["normw"] = nc.dram_tensor("normw", (128, 2), F32, kind="ExternalInput").ap()
    # causal mask strip [128, 896]: mask[kl, c] = 1.0 if kl <= c - 384
    d["maskb"] = nc.dram_tensor("maskb", (128, 896), F32R,
                                kind="ExternalInput").ap()
    outT = nc.dram_tensor("outT", (D, T), F32, kind="ExternalOutput").ap()

    inv_sqrt_hd = 1.0 / np.sqrt(HD)

    with tile.TileContext(nc) as tc:
        with (
            tc.tile_pool(name="consts", bufs=1) as consts,
            tc.tile_pool(name="raw", bufs=1) as rawp,
            tc.tile_pool(name="wqkv", bufs=1) as wqkvp,
            tc.tile_pool(name="final", bufs=1) as finalp,
            tc.tile_pool(name="xblk", bufs=2) as xp,
            tc.tile_pool(name="scratch", bufs=2) as scr,
            tc.tile_pool(name="exp", bufs=3) as expp,
            tc.tile_pool(name="attn", bufs=3) as attnp,
            tc.tile_pool(name="ostage", bufs=3) as ostp,
            tc.tile_pool(name="wo", bufs=2) as wop,
            tc.tile_pool(name="psacc", bufs=4, space="PSUM") as psacc,
            tc.tile_pool(name="pssum", bufs=1, space="PSUM") as pssum,
            tc.tile_pool(name="pssm", bufs=3, space="PSUM") as pssm,
        ):
            # ---- constants ----
            # cosT rows 0:64; ssin2: rows 0:32 = +sin, rows 32:64 = -sin
            # (both tiles base-partition 0 so 2-input DVE ops stay aligned)
            cosT_t = consts.tile([64, T], F32)
            nc.scalar.dma_start(cosT_t, d["trig"][0:64])
            ssin2_t = consts.tile([64, T], F32)
            nc.scalar.dma_start(ssin2_t, d["trig"][64:128])
            convw_t = consts.tile([128, 3, 2, KCONV], F32)
            nc.sync.dma_start(convw_t, d["convw"])
            normw_t = consts.tile([128, 2], F32)
            nc.sync.dma_start(normw_t, d["normw"])
            mask_t = consts.tile([128, 896], F32R)
            nc.scalar.dma_start(mask_t, d["maskb"])
            ones_f = consts.tile([128, 1], F32)
            nc.vector.memset(ones_f, 1.0)
            ones_hd = consts.tile([128, 1], F32R)   # lhsT for partition sums
            nc.vector.tensor_copy(ones_hd, ones_f)
            ones_1f = consts.tile([1, 128], F32)
            nc.vector.memset(ones_1f, 1.0)
            ones_1 = consts.tile([1, 128], F32R)    # lhsT for bcast over parts
            nc.vector.tensor_copy(ones_1, ones_1f)
            ident_f = consts.tile([128, 128], F32)
            make_identity(nc, ident_f)
            ident = consts.tile([128, 128], F32R)
            nc.vector.tensor_copy(ident, ident_f)
            eps_t = consts.tile([1, 1], F32)
            nc.vector.memset(eps_t, EPS)

            # ---- persistent buffers ----
            # raw (pre-conv) projections, padded by PAD zero cols at left
            rawq = rawp.tile([128, MPC, T + PAD], BF16)
            rawk = rawp.tile([128, MPC, T + PAD], BF16)
            rawv = rawp.tile([128, MPC, T + PAD], BF16)
            for r in (rawq, rawk, rawv):
                nc.vector.memset(r[:, :, 0:PAD], 0.0)
            # final q/k in head-transposed layout [HD, m, T]
            qfT = finalp.tile([128, MPC, T], F32R)
            kfT = finalp.tile([128, MPC, T], F32R)
            # v in natural layout per key-chunk: [t(128), m, chunk, HD]
            vtr = finalp.tile([128, MPC, NKC, HD], F32R)

            # =============== Phase bodies (emitted software-pipelined) ====
            # A(t): QKV projection for q-tile t.  B(s): conv/silu/rms/rope
            # for slice s (needs A(s) only, thanks to the causal pad).
            # C(t): attention + output projection for q-tile t (needs B(<=t)).
            w_all = wqkvp.tile([128, KD, 3, CPC], BF16)
            raws = (rawq, rawk, rawv)
            groups = [[(0, 0), (0, 1), (1, 0)], [(1, 1), (2, 0), (2, 1)]]

            def phaseA(tq, first=False):
                xb = xp.tile([128, KD, NT], BF16, name="xb")
                for k in range(KD):
                    if first:  # interleave weight-chunk loads with x(0)
                        for pi, wd in enumerate((d["wqT"], d["wkT"],
                                                 d["wvT"])):
                            deng = nc.sync if (k * 3 + pi) % 2 == 0 \
                                else nc.scalar
                            deng.dma_start(
                                w_all[:, k, pi, :],
                                wd[k * 128:(k + 1) * 128, :])
                    deng = nc.sync if k % 2 == 0 else nc.scalar
                    deng.dma_start(
                        xb[:, k, :],
                        d["xT"][k * 128:(k + 1) * 128,
                                tq * NT:(tq + 1) * NT],
                    )
                for grp in groups:
                    pst = [psacc.tile([128, NT], F32, tag="acc",
                                      name=f"acc{gi}")
                           for gi in range(3)]
                    for k in range(KD):
                        for gi, (pi, m) in enumerate(grp):
                            nc.tensor.matmul(
                                pst[gi],
                                w_all[:, k, pi, m * 128:(m + 1) * 128],
                                xb[:, k, :],
                                start=(k == 0),
                                stop=(k == KD - 1),
                            )
                    for gi, (pi, m) in enumerate(grp):
                        dst = raws[pi][:, m,
                                       PAD + tq * NT:PAD + (tq + 1) * NT]
                        nc.vector.tensor_copy(dst, pst[gi])

            def conv4(raw, pi, m, s):
                """4-tap causal depthwise conv on a 512-slice -> f32 scratch."""
                base = s * NT
                t0 = scr.tile([128, NT], F32, tag="cvA", name="cv0")
                nc.vector.tensor_scalar_mul(
                    t0, raw[:, m, base:base + NT], convw_t[:, pi, m, 0:1]
                )
                for j in (1, 2, 3):
                    t1 = scr.tile([128, NT], F32, tag=("cvB", "cvA")[j % 2],
                                  name="cvj")
                    nc.vector.scalar_tensor_tensor(
                        t1, raw[:, m, base + j:base + j + NT],
                        convw_t[:, pi, m, j:j + 1], t0,
                        mybir.AluOpType.mult, mybir.AluOpType.add,
                    )
                    t0 = t1
                return t0

            def phaseB(s):
                sl = slice(s * NT, (s + 1) * NT)
                for m in range(MPC):
                    # ---- q and k: conv, silu, rms-norm, rope ----
                    for pi, raw, fin, nwi in ((0, rawq, qfT, 0),
                                              (1, rawk, kfT, 1)):
                        cv = conv4(raw, pi, m, s)
                        sv = scr.tile([128, NT], F32, tag="silu")
                        nc.scalar.activation(
                            sv, cv, mybir.ActivationFunctionType.Silu)
                        sq = scr.tile([128, NT], F32R, tag="sq")
                        nc.scalar.activation(
                            sq, sv, mybir.ActivationFunctionType.Square)
                        ps_ss = pssm.tile([1, NT], F32, tag="sm")
                        nc.tensor.matmul(ps_ss, ones_hd, sq,
                                         start=True, stop=True)
                        rstd = scr.tile([1, NT], F32, tag="rst", name="rstd")
                        nc.scalar.activation(
                            rstd, ps_ss, mybir.ActivationFunctionType.Sqrt,
                            scale=1.0 / HD, bias=eps_t)
                        rr = scr.tile([1, NT], F32, tag="rst", name="rr")
                        nc.vector.reciprocal_approx_fast(rr, rstd)
                        ps_rb = pssm.tile([128, NT], F32, tag="sm")
                        nc.tensor.matmul(ps_rb, ones_1f, rr,
                                         start=True, stop=True)
                        qn = sv
                        nc.vector.scalar_tensor_tensor(
                            qn, sv, normw_t[:, nwi:nwi + 1], ps_rb,
                            mybir.AluOpType.mult, mybir.AluOpType.mult,
                        )
                        # rope rows 0:RD (pass-through rows RD:128):
                        # rot2[:,0] = qn_rot*cos; rot2[:,1] = rotate_half(qn)
                        # * sign-folded sin via output-offset muls.
                        rot2 = scr.tile([64, 2, NT], F32, tag="rot2")
                        nc.gpsimd.tensor_mul(rot2[0:32, 1, :], qn[32:64],
                                             ssin2_t[32:64, sl])
                        nc.gpsimd.tensor_mul(rot2[32:64, 1, :], qn[0:32],
                                             ssin2_t[0:32, sl])
                        nc.vector.tensor_mul(rot2[:, 0, :], qn[0:RD],
                                             cosT_t[:, sl])
                        nc.gpsimd.tensor_add(fin[0:RD, m, sl], rot2[:, 0, :],
                                             rot2[:, 1, :])
                        nc.scalar.copy(fin[RD:128, m, sl], qn[RD:128])
                    # ---- v: conv, silu, transpose to natural layout ----
                    cv = conv4(rawv, 2, m, s)
                    vv = scr.tile([128, NT], F32R, tag="gvB", name="vv")
                    nc.scalar.activation(
                        vv, cv, mybir.ActivationFunctionType.Silu)
                    ps_tr = pssm.tile([128, NT], F32R, tag="sm")
                    for sub in range(NT // 128):
                        nc.tensor.transpose(
                            ps_tr[:, sub * 128:(sub + 1) * 128],
                            vv[:, sub * 128:(sub + 1) * 128], ident)
                    nc.scalar.copy(
                        vtr[:, m, s * (NT // 128):(s + 1) * (NT // 128), :],
                        ps_tr.rearrange("p (s h) -> p s h", h=128))

            def phaseC(tq):
                qsl = slice(tq * NT, (tq + 1) * NT)
                attn_m = []
                for m in range(MPC):
                    nch = 4 * tq + 4
                    ps_attn = psacc.tile([128, NT], F32, tag="acc",
                                         name="ps_attn")
                    ps_sum = pssum.tile([1, NT], F32, tag="sum1",
                                        name="ps_sum")

                    def qk(tk):
                        ps_s = pssm.tile([128, NT], F32, tag="sm",
                                         name="ps_s")
                        nc.tensor.matmul(
                            ps_s, kfT[:, m, tk * 128:(tk + 1) * 128],
                            qfT[:, m, qsl], start=True, stop=True)
                        e = expp.tile([128, NT], F32R, tag="e", name="e")
                        nc.scalar.activation(
                            e, ps_s, mybir.ActivationFunctionType.Exp,
                            scale=inv_sqrt_hd)
                        dd = tk * 128 - tq * NT
                        if dd >= 0:  # diagonal chunk: causal mask
                            nc.vector.tensor_mul(
                                e, e, mask_t[:, 384 - dd:896 - dd])
                        return e

                    # software-pipeline QK ahead of PV by two chunks
                    epipe = [qk(t) for t in range(min(2, nch))]
                    for tk in range(nch):
                        if tk + 2 < nch:
                            epipe.append(qk(tk + 2))
                        e = epipe.pop(0)
                        nc.tensor.matmul(
                            ps_attn, vtr[:, m, tk, :], e,
                            start=(tk == 0), stop=(tk == nch - 1))
                        nc.tensor.matmul(
                            ps_sum, ones_hd, e,
                            start=(tk == 0), stop=(tk == nch - 1))
                    # normalize: attn^T *= 1/sumexp (broadcast over parts)
                    rr = scr.tile([1, NT], F32, tag="rst", name="rrs")
                    nc.vector.reciprocal_approx_fast(rr, ps_sum)
                    ps_rb = pssm.tile([128, NT], F32, tag="sm", name="ps_rb")
                    nc.tensor.matmul(ps_rb, ones_1f, rr, start=True,
                                     stop=True)
                    rb = scr.tile([128, NT], F32, tag="rbs")
                    nc.scalar.copy(rb, ps_rb)
                    am = attnp.tile([128, NT], F32R, tag="am", name="am")
                    nc.vector.tensor_mul(am, ps_attn, rb)
                    attn_m.append(am)
                # output projection for this q tile (wo prefetch 2 ahead)
                def wo_load(i):
                    wo_ch = wop.tile([128, 2, 128], F32R, tag="wo",
                                     name="wo_ch")
                    nc.sync.dma_start(
                        wo_ch,
                        d["woT"][:, i * 128:(i + 1) * 128].rearrange(
                            "(j p) n -> p j n", p=128))
                    return wo_ch
                wopipe = [wo_load(0), wo_load(1)]
                for i in range(D // 128):
                    if i + 2 < D // 128:
                        wopipe.append(wo_load(i + 2))
                    wo_ch = wopipe.pop(0)
                    ps_o = psacc.tile([128, NT], F32, tag="acc", name="ps_o")
                    for j in range(MPC):
                        nc.tensor.matmul(ps_o, wo_ch[:, j, :], attn_m[j],
                                         start=(j == 0), stop=(j == MPC - 1))
                    ost = ostp.tile([128, NT], F32, tag="ost", name="ost")
                    nc.vector.tensor_copy(ost, ps_o)
                    nc.sync.dma_start(outT[i * 128:(i + 1) * 128, qsl], ost)

            # pipelined emission: A two tiles ahead of B/C
            phaseA(0, first=True)
            phaseA(1)
            for t in range(NQ):
                phaseB(t)
                phaseC(t)
                if t + 2 < NQ:
                    phaseA(t + 2)

    nc.compile()
    return nc


def _prep_inputs(hidden_states, cos, sin, Wq, Wk, Wv, Wo,
                 conv_q_w, conv_k_w, conv_v_w, q_norm_w, k_norm_w):
    f = np.float32
    bf = ml_dtypes.bfloat16
    x = np.asarray(hidden_states, f)[0]            # [T, D]
    xT = np.ascontiguousarray(x.T.astype(bf))      # [D, T] bf16
    WqT = np.ascontiguousarray(np.asarray(Wq, f).T.astype(bf))
    WkT = np.ascontiguousarray(np.asarray(Wk, f).T.astype(bf))
    WvT = np.ascontiguousarray(np.asarray(Wv, f).T.astype(bf))
    WoT = np.ascontiguousarray(np.asarray(Wo, f).T)

    cosT = np.asarray(cos, f)[0].T                 # [RD, T]
    sinT = np.asarray(sin, f)[0].T
    trig = np.zeros((128, T), f)
    trig[0:RD] = cosT
    # ssin2 block (device rows 0:64): [0:32] = +sin[32:64], [32:64] = -sin[0:32]
    trig[RD:RD + 32] = sinT[32:64]
    trig[RD + 32:2 * RD] = -sinT[0:32]

    # causal mask strip: mask[kl, c] = 1.0 iff kl <= c - 384
    kl = np.arange(128, dtype=f)[:, None]
    cc = np.arange(896, dtype=f)[None, :]
    maskb = (kl <= cc - 384).astype(f)

    nw = np.zeros((128, 2), f)
    nw[:, 0] = np.asarray(q_norm_w, f)
    nw[:, 1] = np.asarray(k_norm_w, f)

    in_maps = []
    for c in range(NCORES):
        sl = slice(c * CPC, (c + 1) * CPC)
        convw = np.zeros((128, 3, 2, KCONV), f)
        for pi, cw in enumerate((conv_q_w, conv_k_w, conv_v_w)):
            convw[:, pi] = np.asarray(cw, f)[sl].reshape(MPC, 128, KCONV
                                                         ).transpose(1, 0, 2)
        in_maps.append({
            "xT": xT,
            "wqT": np.ascontiguousarray(WqT[:, sl]),
            "wkT": np.ascontiguousarray(WkT[:, sl]),
            "wvT": np.ascontiguousarray(WvT[:, sl]),
            "woT": np.ascontiguousarray(WoT[sl, :]),
            "trig": trig,
            "convw": np.ascontiguousarray(convw),
            "normw": nw,
            "maskb": maskb,
        })
    return in_maps


def kernel(hidden_states, cos, sin, Wq, Wk, Wv, Wo,
           conv_q_w, conv_k_w, conv_v_w, q_norm_w, k_norm_w,
           _trace=False):
    global _COMPILED
    if _COMPILED is None:
        _COMPILED = _build()
    nc = _COMPILED
    in_maps = _prep_inputs(hidden_states, cos, sin, Wq, Wk, Wv, Wo,
                           conv_q_w, conv_k_w, conv_v_w, q_norm_w, k_norm_w)
    res = bass_utils.run_bass_kernel_spmd(
        nc, in_maps, core_ids=list(range(NCORES)), trace=_trace)
    acc = np.zeros((D, T), np.float64)
    for r in res.results:
        acc += r["outT"]
    out = np.ascontiguousarray(acc.T.astype(np.float32))[None]
    if _trace:
        kernel._last_results = res
    return out



# revision 15
# speedup vs baseline: 1.2322x; 1.2322x over previous
"""Trainium2 Bass kernel for nn_Attention_34033320854122.

Dense transformer attention block: QKV proj -> causal depthwise conv+SiLU ->
per-head RMSNorm -> partial RoPE -> causal attention -> output projection.

Sharding: tensor-parallel over the 16 heads across 8 NeuronCores (2 heads =
256 channels per core). Each core computes q/k/v for its channels (full
contraction over D), runs attention for its 2 heads, and produces a partial
output projection (outT_partial = Wo[:, cols] @ attn_cols^T). The host sums
the 8 partials and transposes.

v2 design notes (vs the 430us baseline):
- Everything resident in SBUF: xT (bf16, 64KiB/part), QKV weights, Wo.
  No steady-state input DMA -> no PE stalls on DMA.
- bf16 on all non-PSUM paths (conv, rope, q/k/v finals, exp(e), am, wo):
  DVE gets its 2x 2-byte fast mode, DMA bytes halve. PE matmul rate is
  identical for bf16 and f32r (1 row/cycle), fp32 would be 4x slower.
- RoPE runs on DVE via stream_shuffle. Host permutes q/k rot channels in
  quadrant-interleaved order so each RoPE partner (c <-> c+32) lives in the
  same 32-partition quadrant, which is the only shuffle the DVE supports.
  Per head the "A" tile stacks q-rot(64) over k-rot(64) channels so all
  rope DVE ops are full 128-partition ops; the "B" tile holds pass-through
  channels and needs no rope at all.
- Scores stay in [key, query] layout; softmax denominator via ones-matmul
  on PE, normalization broadcast via f32r matmul (baseline used fp32 = 4x).
- Emission is woven: C(t) attention ⋈ B(t+1) conv/norm/rope ⋈ A(t+2)
  projection so the PE instruction queue never starves. TRN2's PE drops
  from 2.4GHz to 1.2GHz for 3us after ANY idle gap, so PE continuity is
  worth more than any single op count.

Numerics: RoPE's global negation of the rotated sub-dim cancels in q.k and
is skipped. Softmax runs without max-subtraction (scores are O(1) bounded).
"""

import ml_dtypes
import numpy as np

import concourse.bacc as bacc
import concourse.tile as tile
import concourse.mybir as mybir
from concourse import bass_utils
from concourse.masks import make_identity

# Problem shape (hardcoded per contract)
B, T, D = 1, 2048, 2048
H, HD = 16, 128
RD = 64
KCONV = 4
EPS = 1e-5
NCORES = 8
CPC = D // NCORES      # channels per core = 256
MPC = CPC // HD        # heads per core = 2
NT = 512               # q-tile / moving dim
NQ = T // NT           # 4 q tiles
KD = D // 128          # 16 contraction chunks
PAD = KCONV - 1        # causal conv history
NBLK = 6               # A0 B0 V0 A1 B1 V1

F32 = mybir.dt.float32
F32R = mybir.dt.float32r
BF16 = mybir.dt.bfloat16

_COMPILED = None

# quadrant-interleaved rot-channel permutation: position p holds channel
# PI_ROT[p]; rope partner (c +/- 32) sits in the same 32-partition quadrant
# 16 rows away, reachable by stream_shuffle.
PI_ROT = np.zeros(64, np.int64)
for _p in range(64):
    _q, _r = divmod(_p, 32)
    PI_ROT[_p] = (_q * 16 + _r) if _r < 16 else (32 + _q * 16 + _r - 16)
SHUF_MASK = [(i + 16) % 32 for i in range(32)]


def _build():
    nc = bacc.Bacc("TRN2", target_bir_lowering=False, debug=False,
                   num_devices=NCORES)

    d = {}
    d["xT"] = nc.dram_tensor("xT", (D, T), BF16, kind="ExternalInput").ap()
    # QKV weights, block layout: cols = blk*128+j, blocks [A0 B0 V0 A1 B1 V1]
    d["wab"] = nc.dram_tensor("wab", (D, NBLK * 128), BF16,
                              kind="ExternalInput").ap()
    d["wo"] = nc.dram_tensor("wo", (CPC, D), BF16, kind="ExternalInput").ap()
    # trig[:,0,:] = cos2 (per permuted channel, stacked q|k), [:,1,:] = ss2
    d["trig"] = nc.dram_tensor("trig", (128, 2, T), BF16,
                               kind="ExternalInput").ap()
    d["convw"] = nc.dram_tensor("convw", (128, NBLK, KCONV), F32,
                                kind="ExternalInput").ap()
    d["normw"] = nc.dram_tensor("normw", (128, 4), F32,
                                kind="ExternalInput").ap()
    # causal mask strip: mask[kl, c] = 1.0 iff kl <= c - 384
    d["maskb"] = nc.dram_tensor("maskb", (128, 896), BF16,
                                kind="ExternalInput").ap()
    # small selector constants (host-baked; partition-offset memsets are
    # rejected by the BIR verifier)
    d["b1d"] = nc.dram_tensor("b1d", (2, 128), F32R,
                              kind="ExternalInput").ap()
    d["ones2d"] = nc.dram_tensor("ones2d", (128, 2), BF16,
                                 kind="ExternalInput").ap()
    outT = nc.dram_tensor("outT", (D, T), BF16, kind="ExternalOutput").ap()

    inv_sqrt_hd = 1.0 / np.sqrt(HD)

    with tile.TileContext(nc) as tc:
        with (
            tc.tile_pool(name="consts", bufs=1) as consts,
            tc.tile_pool(name="resid", bufs=1) as resid,
            tc.tile_pool(name="persist", bufs=1) as persist,
            tc.tile_pool(name="scr", bufs=2) as scr,
            tc.tile_pool(name="expp", bufs=3) as expp,
            tc.tile_pool(name="attnp", bufs=3) as attnp,
            tc.tile_pool(name="ostg", bufs=4) as ostg,
            tc.tile_pool(name="psacc", bufs=2, space="PSUM") as psacc,
            tc.tile_pool(name="psattn", bufs=2, space="PSUM") as psattn,
            tc.tile_pool(name="pssum", bufs=1, space="PSUM") as pssum,
            tc.tile_pool(name="psqk", bufs=2, space="PSUM") as psqk,
            tc.tile_pool(name="psmisc", bufs=1, space="PSUM") as psmisc,
        ):
            # ---------------- constants ----------------
            trig_sb = consts.tile([128, 2, T], BF16)
            nc.scalar.dma_start(trig_sb, d["trig"])
            convw_t = consts.tile([128, NBLK, KCONV], F32)
            nc.scalar.dma_start(convw_t, d["convw"])
            normw_t = consts.tile([128, 4], F32)
            nc.scalar.dma_start(normw_t, d["normw"])
            maskb_t = consts.tile([128, 896], BF16)
            nc.scalar.dma_start(maskb_t, d["maskb"])

            ones2 = consts.tile([128, 2], BF16)     # lhsT: rms sumsq selector
            nc.scalar.dma_start(ones2, d["ones2d"])
            b1 = consts.tile([2, 128], F32R)        # lhsT: rstd bcast selector
            nc.scalar.dma_start(b1, d["b1d"])       # raw f32 bits == f32r
            ones_hd = consts.tile([128, 1], BF16)   # lhsT: softmax denominator
            nc.vector.memset(ones_hd, 1.0)
            ones_1f = consts.tile([1, 128], F32)
            nc.vector.memset(ones_1f, 1.0)
            ones_1 = consts.tile([1, 128], F32R)    # lhsT: bcast over parts
            nc.vector.tensor_copy(ones_1, ones_1f)
            eps2 = consts.tile([2, 1], F32)
            nc.vector.memset(eps2, EPS)
            ident_f = consts.tile([128, 128], F32)
            make_identity(nc, ident_f)
            ident = consts.tile([128, 128], BF16)
            nc.vector.tensor_copy(ident, ident_f)

            # ---------------- resident tensors ----------------
            xT_sb = resid.tile([128, KD, T], BF16)
            w_sb = resid.tile([128, KD, NBLK, 128], BF16)
            wo_sb = resid.tile([128, MPC, D], BF16)

            # startup DMA: slice-0 x and all weights first (A(0) needs them)
            for k in range(KD):
                deng = nc.gpsimd if k % 2 == 0 else nc.scalar
                deng.dma_start(w_sb[:, k, :, :],
                               d["wab"][k * 128:(k + 1) * 128].rearrange(
                                   "p (b n) -> p b n", n=128))
                nc.sync.dma_start(xT_sb[:, k, 0:NT],
                                  d["xT"][k * 128:(k + 1) * 128, 0:NT])
            for m in range(MPC):
                nc.gpsimd.dma_start(wo_sb[:, m, :],
                                    d["wo"][m * 128:(m + 1) * 128, :])

            # ---------------- persistent intermediates ----------------
            raw = persist.tile([128, NBLK, T + PAD], BF16)
            nc.vector.memset(raw[:, :, 0:PAD], 0.0)
            qfT = persist.tile([128, MPC, T], BF16)
            kfT = persist.tile([128, MPC, T], BF16)
            vtr = persist.tile([128, MPC, KD, HD], BF16)

            # remaining x slices stream in during A(0)/A(1)
            def emit_x_rest():
                for s in range(1, NQ):
                    for k in range(KD):
                        deng = nc.sync if k % 2 == 0 else nc.scalar
                        deng.dma_start(
                            xT_sb[:, k, s * NT:(s + 1) * NT],
                            d["xT"][k * 128:(k + 1) * 128,
                                    s * NT:(s + 1) * NT])

            # ---------------- phase A: QKV projection ----------------
            def A_units(t):
                """24 quarter-block units (4 matmuls each) + evac closures."""
                units = []
                state = {}

                def quarter(b, qi):
                    def emit():
                        if qi == 0:
                            state[b] = psacc.tile([128, NT], F32, tag="acc",
                                                  name=f"acc{t}_{b}")
                        ps = state[b]
                        for k in range(qi * 4, qi * 4 + 4):
                            nc.tensor.matmul(
                                ps, w_sb[:, k, b, :],
                                xT_sb[:, k, t * NT:(t + 1) * NT],
                                start=(k == 0), stop=(k == KD - 1))
                        if qi == 3:
                            dst = raw[:, b, PAD + t * NT:PAD + (t + 1) * NT]
                            if b % 2 == 0:
                                nc.scalar.copy(dst, ps)
                            else:
                                nc.vector.tensor_copy(dst, ps)
                    return emit
                for b in range(NBLK):
                    for qi in range(4):
                        units.append(quarter(b, qi))
                return units

            # ---------------- phase B: conv/silu/rms/rope ----------------
            def conv4(blk, t, out_dt=BF16, tagp="cv"):
                base = t * NT
                t0 = scr.tile([128, NT], out_dt, tag=tagp + "A", name="cv0")
                nc.vector.tensor_scalar_mul(
                    t0, raw[:, blk, base:base + NT], convw_t[:, blk, 0:1])
                for j in (1, 2, 3):
                    t1 = scr.tile([128, NT], out_dt,
                                  tag=tagp + ("B", "A")[j % 2], name="cvj")
                    nc.vector.scalar_tensor_tensor(
                        t1, raw[:, blk, base + j:base + j + NT],
                        convw_t[:, blk, j:j + 1], t0,
                        mybir.AluOpType.mult, mybir.AluOpType.add)
                    t0 = t1
                return t0

            def B_units(t, m):
                """Emission closures for head m of tile t.
                returns (front, mid, back): front = DVE conv+silu+sq,
                mid = PE rms bits (needs front), back = qn/rope/V."""
                sl = slice(t * NT, (t + 1) * NT)
                bA, bB, bV = 3 * m, 3 * m + 1, 3 * m + 2
                st = {}

                def front():
                    cvA = conv4(bA, t)
                    svA = scr.tile([128, NT], BF16, tag="sv", name="svA")
                    nc.scalar.activation(svA, cvA,
                                         mybir.ActivationFunctionType.Silu)
                    cvB = conv4(bB, t)
                    svB = scr.tile([128, NT], BF16, tag="sv", name="svB")
                    nc.scalar.activation(svB, cvB,
                                         mybir.ActivationFunctionType.Silu)
                    sqA = scr.tile([128, NT], BF16, tag="sq", name="sqA")
                    nc.vector.tensor_tensor(sqA, svA, svA,
                                            mybir.AluOpType.mult)
                    sqB = scr.tile([128, NT], BF16, tag="sq", name="sqB")
                    nc.vector.tensor_tensor(sqB, svB, svB,
                                            mybir.AluOpType.mult)
                    st.update(svA=svA, svB=svB, sqA=sqA, sqB=sqB)

                def mid():
                    # PE: sum of squares over q/k channel halves
                    ps_ss = psmisc.tile([2, NT], F32, tag="sm", name="ps_ss")
                    nc.tensor.matmul(ps_ss, ones2, st["sqA"],
                                     start=True, stop=False)
                    nc.tensor.matmul(ps_ss, ones2, st["sqB"],
                                     start=False, stop=True)
                    rstd = scr.tile([2, NT], F32, tag="rst", name="rstd")
                    nc.scalar.activation(
                        rstd, ps_ss, mybir.ActivationFunctionType.Sqrt,
                        scale=1.0 / HD, bias=eps2)
                    rr2f = scr.tile([2, NT], F32, tag="rsg", name="rr2f")
                    nc.vector.reciprocal_approx_fast(rr2f, rstd)
                    rr2 = scr.tile([2, NT], F32R, tag="rst", name="rr2")
                    nc.vector.tensor_copy(rr2, rr2f)
                    ps_rb = psmisc.tile([128, NT], F32, tag="sm",
                                        name="ps_rb")
                    nc.tensor.matmul(ps_rb, b1, rr2, start=True, stop=True)
                    rb_sb = scr.tile([128, NT], BF16, tag="rb", name="rb_sb")
                    nc.scalar.copy(rb_sb, ps_rb)
                    st["rb"] = rb_sb

                def back():
                    rb_sb = st["rb"]
                    qnA = scr.tile([128, NT], BF16, tag="qn", name="qnA")
                    nc.vector.scalar_tensor_tensor(
                        qnA, st["svA"], normw_t[:, 2 * m:2 * m + 1], rb_sb,
                        mybir.AluOpType.mult, mybir.AluOpType.mult)
                    qnB = scr.tile([128, NT], BF16, tag="qn", name="qnB")
                    nc.vector.scalar_tensor_tensor(
                        qnB, st["svB"], normw_t[:, 2 * m + 1:2 * m + 2],
                        rb_sb, mybir.AluOpType.mult, mybir.AluOpType.mult)
                    # pass-through channels: scatter to q/k finals
                    nc.gpsimd.dma_start(qfT[64:128, m, sl], qnB[0:64, :])
                    nc.gpsimd.dma_start(kfT[64:128, m, sl], qnB[64:128, :])
                    # rope: partner swap within quadrants, then 3 DVE muls
                    shf = scr.tile([128, NT], BF16, tag="shf", name="shf")
                    nc.vector.stream_shuffle(shf, qnA, SHUF_MASK)
                    t1 = scr.tile([128, NT], BF16, tag="ro", name="ro1")
                    nc.vector.tensor_tensor(t1, qnA, trig_sb[:, 0, sl],
                                            mybir.AluOpType.mult)
                    nc.vector.tensor_tensor(shf, shf, trig_sb[:, 1, sl],
                                            mybir.AluOpType.mult)
                    nc.vector.tensor_tensor(t1, t1, shf,
                                            mybir.AluOpType.add)
                    nc.gpsimd.dma_start(qfT[0:64, m, sl], t1[0:64, :])
                    nc.gpsimd.dma_start(kfT[0:64, m, sl], t1[64:128, :])

                def vwork():
                    cvV = conv4(bV, t, tagp="cw")
                    svV = scr.tile([128, NT], BF16, tag="svv", name="svV")
                    nc.scalar.activation(svV, cvV,
                                         mybir.ActivationFunctionType.Silu)
                    ps_tr = psmisc.tile([128, NT], BF16, tag="sm",
                                        name="ps_tr")
                    for sub in range(NT // 128):
                        nc.tensor.transpose(
                            ps_tr[:, sub * 128:(sub + 1) * 128],
                            svV[:, sub * 128:(sub + 1) * 128], ident)
                    nc.scalar.copy(
                        vtr[:, m, t * (NT // 128):(t + 1) * (NT // 128), :],
                        ps_tr.rearrange("p (s h) -> p s h", h=128))

                return front, mid, back, vwork

            # ---------------- phase C: attention + out-proj ----------------
            def C_phase(t, m, fillers):
                """Attention for (tile t, head m). fillers: list of (pos,
                closure) inserted after trio index pos. Returns am tile."""
                nch = 4 * t + 4
                qsl = slice(t * NT, (t + 1) * NT)
                ps_attn = psattn.tile([128, NT], F32, tag="pat",
                                      name="ps_attn")
                ps_sum = pssum.tile([1, NT], F32, tag="sum1", name="ps_sum")
                fmap = {}
                for pos, fn in fillers:
                    fmap.setdefault(min(pos, nch - 1), []).append(fn)

                def qk(ci):
                    ps_s = psqk.tile([128, NT], F32, tag="qk", name="ps_s")
                    nc.tensor.matmul(
                        ps_s, kfT[:, m, ci * 128:(ci + 1) * 128],
                        qfT[:, m, qsl], start=True, stop=True)
                    e = expp.tile([128, NT], BF16, tag="e", name="e")
                    nc.scalar.activation(
                        e, ps_s, mybir.ActivationFunctionType.Exp,
                        scale=inv_sqrt_hd)
                    dd = ci * 128 - t * NT
                    if dd >= 0:  # diagonal chunk: causal mask
                        nc.vector.tensor_tensor(
                            e, e, maskb_t[:, 384 - dd:896 - dd],
                            mybir.AluOpType.mult)
                    return e

                epipe = [qk(ci) for ci in range(min(2, nch))]
                for ci in range(nch):
                    if ci + 2 < nch:
                        epipe.append(qk(ci + 2))
                    e = epipe.pop(0)
                    nc.tensor.matmul(ps_attn, vtr[:, m, ci, :], e,
                                     start=(ci == 0), stop=(ci == nch - 1))
                    nc.tensor.matmul(ps_sum, ones_hd, e,
                                     start=(ci == 0), stop=(ci == nch - 1))
                    for fn in fmap.get(ci, ()):
                        fn()
                rrf = scr.tile([1, NT], F32, tag="rrg", name="rrf")
                nc.vector.reciprocal_approx_fast(rrf, ps_sum)
                rr = scr.tile([1, NT], F32R, tag="rr1", name="rr")
                nc.vector.tensor_copy(rr, rrf)
                return ps_attn, rr

            def C_norm_finish(ps_attn, rr):
                ps_rb = psmisc.tile([128, NT], F32, tag="sm", name="ps_rb2")
                nc.tensor.matmul(ps_rb, ones_1, rr, start=True, stop=True)
                rb2 = scr.tile([128, NT], BF16, tag="rb", name="rb2")
                nc.scalar.copy(rb2, ps_rb)
                am = attnp.tile([128, NT], BF16, tag="am", name="am")
                nc.vector.tensor_tensor(am, ps_attn, rb2,
                                        mybir.AluOpType.mult)
                return am

            def outproj(t, ams, fillers):
                qsl = slice(t * NT, (t + 1) * NT)
                fq = list(fillers)
                for i in range(D // 128):
                    ps_o = psqk.tile([128, NT], F32, tag="qk", name="ps_o")
                    for j in range(MPC):
                        nc.tensor.matmul(ps_o, wo_sb[:, j,
                                                     i * 128:(i + 1) * 128],
                                         ams[j], start=(j == 0),
                                         stop=(j == MPC - 1))
                    ost = ostg.tile([128, NT], BF16, tag="ost", name="ost")
                    if i % 2 == 0:
                        nc.scalar.copy(ost, ps_o)
                    else:
                        nc.vector.tensor_copy(ost, ps_o)
                    nc.sync.dma_start(outT[i * 128:(i + 1) * 128, qsl], ost)
                    if i % 4 == 3 and fq:
                        fq.pop(0)()
                while fq:
                    fq.pop(0)()

            # ---------------- emission schedule ----------------
            # A(0); B(0)⋈A(1); then slots: C(t) ⋈ B(t+1) ⋈ A(t+2)
            a_queues = {t: A_units(t) for t in range(NQ)}

            for u in a_queues[0]:
                u()
            emit_x_rest()

            # B(0) woven into A(1): front(m) before 2 A-blocks, pe-bits after
            a1 = a_queues[1]
            f0, m0_, b0, v0 = B_units(0, 0)
            f1, m1_, b1_, v1 = B_units(0, 1)
            f0()
            for u in a1[0:8]:
                u()
            m0_(); b0()
            f1()
            for u in a1[8:16]:
                u()
            v0(); m1_(); b1_()
            for u in a1[16:24]:
                u()
            v1()

            for t in range(NQ):
                has_b = t + 1 < NQ
                has_a = t + 2 < NQ
                aq = list(a_queues[t + 2]) if has_a else []
                ai = [0]

                def take_a(n=1):
                    out = []
                    for _ in range(n):
                        if ai[0] < len(aq):
                            out.append(aq[ai[0]])
                            ai[0] += 1
                    return out

                if has_b:
                    Bf0, Bm0, Bb0, Bv0 = B_units(t + 1, 0)
                    Bf1, Bm1, Bb1, Bv1 = B_units(t + 1, 1)
                else:
                    Bf0 = Bm0 = Bb0 = Bv0 = Bf1 = Bm1 = Bb1 = Bv1 = None

                nch = 4 * t + 4
                # ---- head 0 ----
                if Bf0:
                    Bf0()
                fillers = []
                if Bm0:
                    fillers.append((max(2, nch - 2), Bm0))
                for pos in range(3, nch, 2):
                    fillers.append((pos, lambda: [u() for u in take_a(1)]))
                ps_attn0, rr0 = C_phase(t, 0, fillers)
                if Bb0:
                    Bb0()
                for u in take_a(2):
                    u()
                am0 = C_norm_finish(ps_attn0, rr0)
                # ---- head 1 ----
                if Bf1:
                    Bf1()
                fillers = []
                if Bv0:
                    fillers.append((1, Bv0))
                if Bm1:
                    fillers.append((max(2, nch - 2), Bm1))
                for pos in range(3, nch, 2):
                    fillers.append((pos, lambda: [u() for u in take_a(1)]))
                ps_attn1, rr1 = C_phase(t, 1, fillers)
                if Bb1:
                    Bb1()
                for u in take_a(2):
                    u()
                am1 = C_norm_finish(ps_attn1, rr1)
                if Bv1:
                    Bv1()
                # ---- output projection ----
                ofill = [lambda: [u() for u in take_a(1)] for _ in range(4)]
                outproj(t, [am0, am1], ofill)
                while ai[0] < len(aq):
                    aq[ai[0]]()
                    ai[0] += 1

    nc.compile()
    return nc


def _prep_inputs(hidden_states, cos, sin, Wq, Wk, Wv, Wo,
                 conv_q_w, conv_k_w, conv_v_w, q_norm_w, k_norm_w):
    f = np.float32
    bf = ml_dtypes.bfloat16
    x = np.asarray(hidden_states, f)[0]            # [T, D]
    xT = np.ascontiguousarray(x.T.astype(bf))      # [D, T] bf16
    WqT = np.asarray(Wq, f).T                      # [D, D] (col = out ch)
    WkT = np.asarray(Wk, f).T
    WvT = np.asarray(Wv, f).T
    WoT = np.asarray(Wo, f).T                      # [D(in=attn ch), D(out)]

    cosT = np.asarray(cos, f)[0].T                 # [RD, T]
    sinT = np.asarray(sin, f)[0].T

    # trig tables in permuted+stacked layout: part p of the A tile holds
    # channel PI_ROT[p % 64]; ss2 = sign-folded sin for the shuffled partner
    trig = np.zeros((128, 2, T), f)
    for half in range(2):
        for p in range(64):
            c = PI_ROT[p]
            trig[half * 64 + p, 0] = cosT[c]
            trig[half * 64 + p, 1] = (-1.0 if c < 32 else 1.0) * sinT[c]
    trig = trig.astype(bf)

    kl = np.arange(128, dtype=f)[:, None]
    cc = np.arange(896, dtype=f)[None, :]
    maskb = (kl <= cc - 384).astype(bf)

    cw = {0: np.asarray(conv_q_w, f), 1: np.asarray(conv_k_w, f),
          2: np.asarray(conv_v_w, f)}
    qnw = np.asarray(q_norm_w, f)
    knw = np.asarray(k_norm_w, f)

    in_maps = []
    for core in range(NCORES):
        wab = np.zeros((D, NBLK * 128), f)
        convw = np.zeros((128, NBLK, KCONV), f)
        normw = np.zeros((128, 4), f)
        for m in range(MPC):
            h = 2 * core + m
            qch = h * 128 + PI_ROT            # rot q channels (global)
            kch = h * 128 + PI_ROT            # rot k channels
            # block A_m: [q-rot(64) | k-rot(64)], permuted
            bA = 3 * m
            wab[:, bA * 128:bA * 128 + 64] = WqT[:, qch]
            wab[:, bA * 128 + 64:bA * 128 + 128] = WkT[:, kch]
            convw[0:64, bA] = cw[0][qch]
            convw[64:128, bA] = cw[1][kch]
            normw[0:64, 2 * m] = qnw[PI_ROT]
            normw[64:128, 2 * m] = knw[PI_ROT]
            # block B_m: [q-pass(64) | k-pass(64)], natural order
            bB = 3 * m + 1
            pch = h * 128 + 64 + np.arange(64)
            wab[:, bB * 128:bB * 128 + 64] = WqT[:, pch]
            wab[:, bB * 128 + 64:bB * 128 + 128] = WkT[:, pch]
            convw[0:64, bB] = cw[0][pch]
            convw[64:128, bB] = cw[1][pch]
            normw[0:64, 2 * m + 1] = qnw[64:128]
            normw[64:128, 2 * m + 1] = knw[64:128]
            # block V_m: natural
            bV = 3 * m + 2
            vch = h * 128 + np.arange(128)
            wab[:, bV * 128:bV * 128 + 128] = WvT[:, vch]
            convw[:, bV] = cw[2][vch]
        b1d = np.zeros((2, 128), f)
        b1d[0, 0:64] = 1.0
        b1d[1, 64:128] = 1.0
        ones2d = np.zeros((128, 2), f)
        ones2d[0:64, 0] = 1.0
        ones2d[64:128, 1] = 1.0
        sl = slice(core * CPC, (core + 1) * CPC)
        in_maps.append({
            "xT": xT,
            "wab": np.ascontiguousarray(wab.astype(bf)),
            "wo": np.ascontiguousarray(WoT[sl, :].astype(bf)),
            "trig": trig,
            "convw": np.ascontiguousarray(convw),
            "normw": normw,
            "maskb": maskb,
            "b1d": b1d,
            "ones2d": ones2d.astype(bf),
        })
    return in_maps


def kernel(hidden_states, cos, sin, Wq, Wk, Wv, Wo,
           conv_q_w, conv_k_w, conv_v_w, q_norm_w, k_norm_w,
           _trace=False):
    global _COMPILED
    if _COMPILED is None:
        _COMPILED = _build()
    nc = _COMPILED
    in_maps = _prep_inputs(hidden_states, cos, sin, Wq, Wk, Wv, Wo,
                           conv_q_w, conv_k_w, conv_v_w, q_norm_w, k_norm_w)
    res = bass_utils.run_bass_kernel_spmd(
        nc, in_maps, core_ids=list(range(NCORES)), trace=_trace)
    acc = np.zeros((D, T), np.float64)
    for r in res.results:
        acc += r["outT"].astype(np.float64)
    out = np.ascontiguousarray(acc.T.astype(np.float32))[None]
    if _trace:
        kernel._last_results = res
    return out


# revision 34
# speedup vs baseline: 1.4907x; 1.2098x over previous
"""Trainium2 Bass kernel for nn_Attention_34033320854122.

Dense transformer attention block: QKV proj -> causal depthwise conv+SiLU ->
per-head RMSNorm -> partial RoPE -> causal attention -> output projection.

Sharding: tensor-parallel over the 16 heads across 8 NeuronCores (2 heads =
256 channels per core). Each core computes q/k/v for its channels (full
contraction over D), runs attention for its 2 heads, and produces a partial
output projection (outT_partial = Wo[:, cols] @ attn_cols^T). The host sums
the 8 partials and transposes.

v2 design notes (vs the 430us baseline):
- Everything resident in SBUF: xT (bf16, 64KiB/part), QKV weights, Wo.
  No steady-state input DMA -> no PE stalls on DMA.
- bf16 on all non-PSUM paths (conv, rope, q/k/v finals, exp(e), am, wo):
  DVE gets its 2x 2-byte fast mode, DMA bytes halve. PE matmul rate is
  identical for bf16 and f32r (1 row/cycle), fp32 would be 4x slower.
- RoPE runs on DVE via stream_shuffle. Host permutes q/k rot channels in
  quadrant-interleaved order so each RoPE partner (c <-> c+32) lives in the
  same 32-partition quadrant, which is the only shuffle the DVE supports.
  Per head the "A" tile stacks q-rot(64) over k-rot(64) channels so all
  rope DVE ops are full 128-partition ops; the "B" tile holds pass-through
  channels and needs no rope at all.
- Scores stay in [key, query] layout; softmax denominator via ones-matmul
  on PE, normalization broadcast via f32r matmul (baseline used fp32 = 4x).
- Emission is woven: C(t) attention ⋈ B(t+1) conv/norm/rope ⋈ A(t+2)
  projection so the PE instruction queue never starves. TRN2's PE drops
  from 2.4GHz to 1.2GHz for 3us after ANY idle gap, so PE continuity is
  worth more than any single op count.

Numerics: RoPE's global negation of the rotated sub-dim cancels in q.k and
is skipped. Softmax runs without max-subtraction (scores are O(1) bounded).
"""

import ml_dtypes
import numpy as np

import concourse.bacc as bacc
import concourse.tile as tile
import concourse.mybir as mybir
from concourse import bass_utils
from concourse.masks import make_identity

# Problem shape (hardcoded per contract)
B, T, D = 1, 2048, 2048
H, HD = 16, 128
RD = 64
KCONV = 4
EPS = 1e-5
NCORES = 8
CPC = D // NCORES      # channels per core = 256
MPC = CPC // HD        # heads per core = 2
NT = 512               # q-tile / moving dim
NQ = T // NT           # 4 q tiles
KD = D // 128          # 16 contraction chunks
PAD = KCONV - 1        # causal conv history
NBLK = 6               # A0 B0 V0 A1 B1 V1

F32 = mybir.dt.float32
F32R = mybir.dt.float32r
BF16 = mybir.dt.bfloat16

_COMPILED = None

# quadrant-interleaved rot-channel permutation: position p holds channel
# PI_ROT[p]; rope partner (c +/- 32) sits in the same 32-partition quadrant
# 16 rows away, reachable by stream_shuffle.
PI_ROT = np.zeros(64, np.int64)
for _p in range(64):
    _q, _r = divmod(_p, 32)
    PI_ROT[_p] = (_q * 16 + _r) if _r < 16 else (32 + _q * 16 + _r - 16)
SHUF_MASK = [(i + 16) % 32 for i in range(32)]


def _build():
    nc = bacc.Bacc("TRN2", target_bir_lowering=False, debug=False,
                   num_devices=NCORES)

    d = {}
    d["xT"] = nc.dram_tensor("xT", (D, T), BF16, kind="ExternalInput").ap()
    # QKV weights, block layout: cols = blk*128+j, blocks [A0 B0 V0 A1 B1 V1]
    d["wab"] = nc.dram_tensor("wab", (D, NBLK * 128), BF16,
                              kind="ExternalInput").ap()
    d["wo"] = nc.dram_tensor("wo", (CPC, D), BF16, kind="ExternalInput").ap()
    # trig[:,0,:] = cos2 (per permuted channel, stacked q|k), [:,1,:] = ss2
    d["trig"] = nc.dram_tensor("trig", (128, 2, T), BF16,
                               kind="ExternalInput").ap()
    # conv weights pre-scaled by 0.5 (silu(x) = cv + cv*tanh(cv), cv = x/2)
    d["convw"] = nc.dram_tensor("convw", (128, NBLK, KCONV), F32,
                                kind="ExternalInput").ap()
    d["normw"] = nc.dram_tensor("normw", (128, 4), F32,
                                kind="ExternalInput").ap()
    # causal mask strip: mask[kl, c] = 1.0 iff kl <= c - 384
    d["maskb"] = nc.dram_tensor("maskb", (128, 896), BF16,
                                kind="ExternalInput").ap()
    # small selector constants (host-baked; partition-offset memsets are
    # rejected by the BIR verifier).  ones4d cols 0:4 / 4:8 = rms sumsq
    # selectors for m0 / m1 rows of the batched [4,NT] rstd; b4m* are the
    # matching broadcast lhsTs.
    d["ones4d"] = nc.dram_tensor("ones4d", (128, 8), BF16,
                                 kind="ExternalInput").ap()
    d["b4m0"] = nc.dram_tensor("b4m0", (4, 128), F32R,
                               kind="ExternalInput").ap()
    d["b4m1"] = nc.dram_tensor("b4m1", (4, 128), F32R,
                               kind="ExternalInput").ap()
    outT = nc.dram_tensor("outT", (D, T), BF16, kind="ExternalOutput").ap()

    inv_sqrt_hd = 1.0 / np.sqrt(HD)

    with tile.TileContext(nc) as tc:
        with (
            tc.tile_pool(name="consts", bufs=1) as consts,
            tc.tile_pool(name="resid", bufs=1) as resid,
            tc.tile_pool(name="persist", bufs=1) as persist,
            tc.tile_pool(name="scr", bufs=2) as scr,
            tc.tile_pool(name="expp", bufs=3) as expp,
            tc.tile_pool(name="attnp", bufs=2) as attnp,
            tc.tile_pool(name="ostg", bufs=3) as ostg,
            tc.tile_pool(name="psacc", bufs=2, space="PSUM") as psacc,
            tc.tile_pool(name="psattn", bufs=2, space="PSUM") as psattn,
            tc.tile_pool(name="pssum", bufs=1, space="PSUM") as pssum,
            tc.tile_pool(name="psqk", bufs=2, space="PSUM") as psqk,
            tc.tile_pool(name="psmisc", bufs=1, space="PSUM") as psmisc,
        ):
            # ---------------- constants ----------------
            # tiny consts on gpsimd (SWDGE): keeps the HWDGE queues (sync,
            # scalar) free for the big startup x/w transfers
            convw_t = consts.tile([128, NBLK, KCONV], F32)
            nc.scalar.dma_start(convw_t, d["convw"])
            normw_t = consts.tile([128, 4], F32)
            nc.scalar.dma_start(normw_t, d["normw"])
            maskb_t = consts.tile([128, 896], BF16)
            nc.scalar.dma_start(maskb_t, d["maskb"])
            ones4 = consts.tile([128, 8], BF16)     # lhsT: rms sumsq selector
            nc.scalar.dma_start(ones4, d["ones4d"])
            b4m0 = consts.tile([4, 128], F32R)      # lhsT: rstd bcast m0
            nc.scalar.dma_start(b4m0, d["b4m0"])
            b4m1 = consts.tile([4, 128], F32R)      # lhsT: rstd bcast m1
            nc.scalar.dma_start(b4m1, d["b4m1"])
            trig_sb = consts.tile([128, 2, T], BF16)
            nc.scalar.dma_start(trig_sb, d["trig"])

            ones_hd = consts.tile([128, 1], BF16)   # lhsT: softmax denominator
            nc.vector.memset(ones_hd, 1.0)
            ones_1f = consts.tile([1, 128], F32)
            nc.vector.memset(ones_1f, 1.0)
            ones_1 = consts.tile([1, 128], F32R)    # lhsT: bcast over parts
            nc.vector.tensor_copy(ones_1, ones_1f)
            eps4 = consts.tile([4, 1], F32)
            nc.vector.memset(eps4, EPS)
            ident_f = consts.tile([128, 128], F32)
            make_identity(nc, ident_f)
            ident = consts.tile([128, 128], BF16)
            nc.vector.tensor_copy(ident, ident_f)

            # ---------------- resident tensors ----------------
            xT_sb = resid.tile([128, KD, T], BF16)
            w_sb = resid.tile([128, KD, NBLK, 128], BF16)
            wo_sb = resid.tile([128, MPC, D], BF16)

            # startup DMA: slice-0 x and all weights first (A(0) needs them),
            # alternating the two HWDGE queues (sync, scalar)
            for k in range(KD):
                deng, deng2 = (nc.sync, nc.scalar) if k % 2 == 0 \
                    else (nc.scalar, nc.sync)
                deng.dma_start(w_sb[:, k, :, :],
                               d["wab"][k * 128:(k + 1) * 128].rearrange(
                                   "p (b n) -> p b n", n=128))
                deng2.dma_start(xT_sb[:, k, 0:NT],
                                d["xT"][k * 128:(k + 1) * 128, 0:NT])
            for m in range(MPC):
                nc.gpsimd.dma_start(wo_sb[:, m, :],
                                    d["wo"][m * 128:(m + 1) * 128, :])

            # ---------------- persistent intermediates ----------------
            raw = persist.tile([128, NBLK, T + PAD], BF16)
            nc.vector.memset(raw[:, :, 0:PAD], 0.0)
            qfT = persist.tile([128, MPC, T], BF16)
            kfT = persist.tile([128, MPC, T], BF16)
            vtr = persist.tile([128, MPC, KD, HD], BF16)

            # remaining x slices stream in during A(0)/A(1)
            def emit_x_rest():
                for s in range(1, NQ):
                    for k in range(KD):
                        deng = nc.sync if k % 2 == 0 else nc.scalar
                        deng.dma_start(
                            xT_sb[:, k, s * NT:(s + 1) * NT],
                            d["xT"][k * 128:(k + 1) * 128,
                                    s * NT:(s + 1) * NT])

            # ---------------- phase A: QKV projection ----------------
            def A_units(t):
                """24 quarter-block units (4 matmuls each) + evac closures."""
                units = []
                state = {}

                def quarter(b, qi):
                    def emit():
                        if qi == 0:
                            state[b] = psacc.tile([128, NT], F32, tag="acc",
                                                  name=f"acc{t}_{b}")
                        ps = state[b]
                        for k in range(qi * 4, qi * 4 + 4):
                            nc.tensor.matmul(
                                ps, w_sb[:, k, b, :],
                                xT_sb[:, k, t * NT:(t + 1) * NT],
                                start=(k == 0), stop=(k == KD - 1))
                        if qi == 3:
                            dst = raw[:, b, PAD + t * NT:PAD + (t + 1) * NT]
                            if b % 2 == 0:
                                nc.scalar.copy(dst, ps)
                            else:
                                nc.vector.tensor_copy(dst, ps)
                    return emit
                for b in range(NBLK):
                    for qi in range(4):
                        units.append(quarter(b, qi))
                return units

            # ---------------- phase B: conv/silu/rms/rope ----------------
            def conv4(blk, t, tagp="cv"):
                # 4 tensor_scalar muls + 3 adds: all hit the DVE 2-byte
                # fast path (scalar_tensor_tensor does not)
                base = t * NT
                taps = []
                for j in range(KCONV):
                    tj = scr.tile([128, NT], BF16, tag=f"{tagp}{j}",
                                  bufs=2 if j == 0 else 1, name=f"tap{j}")
                    nc.vector.tensor_scalar_mul(
                        tj, raw[:, blk, base + j:base + j + NT],
                        convw_t[:, blk, j:j + 1])
                    taps.append(tj)
                nc.vector.tensor_tensor(taps[0], taps[0], taps[1],
                                        mybir.AluOpType.add)
                nc.vector.tensor_tensor(taps[2], taps[2], taps[3],
                                        mybir.AluOpType.add)
                nc.vector.tensor_tensor(taps[0], taps[0], taps[2],
                                        mybir.AluOpType.add)
                return taps[0]

            def silu2(cv, tag, name):
                """silu from half-scaled conv: sv = (tanh(cv)+1)*cv.
                tanh shares the exp act table -> no table reload."""
                th = scr.tile([128, NT], BF16, tag="tht", name=name + "t")
                nc.scalar.activation(th, cv,
                                     mybir.ActivationFunctionType.Tanh)
                nc.vector.tensor_scalar_add(th, th, 1.0)
                bufs = 4 if tag == "sv" else 2
                sv = scr.tile([128, NT], BF16, tag=tag, bufs=bufs, name=name)
                nc.vector.tensor_tensor(sv, th, cv, mybir.AluOpType.mult)
                return sv

            def B_units(t):
                """Emission closures for tile t (both heads).
                front(m) = DVE conv + tanh-silu + sq; mid = batched rms
                (one sqrt for all 4 rows -> 2 act-table loads per tile);
                back(m) = qn/rope/scatter; vwork(m) = v conv + transpose."""
                sl = slice(t * NT, (t + 1) * NT)
                st = {}

                def front(m):
                    def emit():
                        cvA = conv4(3 * m, t)
                        svA = silu2(cvA, "sv", f"svA{m}")
                        cvB = conv4(3 * m + 1, t)
                        svB = silu2(cvB, "sv", f"svB{m}")
                        sqA = scr.tile([128, NT], BF16, tag="sq", bufs=4,
                                       name="sqA")
                        nc.vector.tensor_tensor(sqA, svA, svA,
                                                mybir.AluOpType.mult)
                        sqB = scr.tile([128, NT], BF16, tag="sq", bufs=4,
                                       name="sqB")
                        nc.vector.tensor_tensor(sqB, svB, svB,
                                                mybir.AluOpType.mult)
                        st[("sv", m)] = (svA, svB)
                        st[("sq", m)] = (sqA, sqB)
                    return emit

                def mid():
                    # batched sum-of-squares: rows 0,1 = m0 q/k, 2,3 = m1
                    ps_ss = psmisc.tile([4, NT], F32, tag="sm", name="ps_ss")
                    tiles = [(st[("sq", 0)][0], 0), (st[("sq", 0)][1], 0),
                             (st[("sq", 1)][0], 1), (st[("sq", 1)][1], 1)]
                    for i, (sq, m) in enumerate(tiles):
                        nc.tensor.matmul(ps_ss, ones4[:, 4 * m:4 * m + 4],
                                         sq, start=(i == 0), stop=(i == 3))
                    rstd = scr.tile([4, NT], F32, tag="rst", bufs=1, name="rstd")
                    nc.scalar.activation(
                        rstd, ps_ss, mybir.ActivationFunctionType.Sqrt,
                        scale=1.0 / HD, bias=eps4)
                    rr4f = scr.tile([4, NT], F32, tag="rsg", bufs=1, name="rr4f")
                    nc.vector.reciprocal_approx_fast(rr4f, rstd)
                    rr4 = scr.tile([4, NT], F32R, tag="rst", bufs=1, name="rr4")
                    nc.vector.tensor_copy(rr4, rr4f)
                    for m, b4 in ((0, b4m0), (1, b4m1)):
                        ps_rb = psmisc.tile([128, NT], F32, tag="sm",
                                            name="ps_rb")
                        nc.tensor.matmul(ps_rb, b4, rr4,
                                         start=True, stop=True)
                        rb_sb = scr.tile([128, NT], BF16, tag="rb", bufs=4,
                                         name="rb_sb")
                        nc.scalar.copy(rb_sb, ps_rb)
                        st[("rb", m)] = rb_sb

                def back(m):
                    def emit():
                        rb_sb = st[("rb", m)]
                        svA, svB = st[("sv", m)]
                        qnA = scr.tile([128, NT], BF16, tag="qn",
                                       name="qnA")
                        nc.vector.scalar_tensor_tensor(
                            qnA, svA, normw_t[:, 2 * m:2 * m + 1], rb_sb,
                            mybir.AluOpType.mult, mybir.AluOpType.mult)
                        qnB = scr.tile([128, NT], BF16, tag="qn",
                                       name="qnB")
                        nc.vector.scalar_tensor_tensor(
                            qnB, svB, normw_t[:, 2 * m + 1:2 * m + 2],
                            rb_sb, mybir.AluOpType.mult,
                            mybir.AluOpType.mult)
                        # pass-through channels: scatter to q/k finals
                        nc.gpsimd.dma_start(qfT[64:128, m, sl], qnB[0:64, :])
                        nc.gpsimd.dma_start(kfT[64:128, m, sl],
                                            qnB[64:128, :])
                        # rope: partner swap within quadrants + 3 DVE muls
                        shf = scr.tile([128, NT], BF16, tag="shf",
                                       name="shf")
                        nc.vector.stream_shuffle(shf, qnA, SHUF_MASK)
                        t1 = scr.tile([128, NT], BF16, tag="ro", name="ro1")
                        nc.vector.tensor_tensor(t1, qnA, trig_sb[:, 0, sl],
                                                mybir.AluOpType.mult)
                        nc.vector.tensor_tensor(shf, shf, trig_sb[:, 1, sl],
                                                mybir.AluOpType.mult)
                        nc.vector.tensor_tensor(t1, t1, shf,
                                                mybir.AluOpType.add)
                        nc.gpsimd.dma_start(qfT[0:64, m, sl], t1[0:64, :])
                        nc.gpsimd.dma_start(kfT[0:64, m, sl],
                                            t1[64:128, :])
                    return emit

                def vwork(m):
                    def emit():
                        cvV = conv4(3 * m + 2, t)
                        svV = silu2(cvV, "svv", f"svV{m}")
                        ps_tr = psmisc.tile([128, NT], BF16, tag="sm",
                                            name="ps_tr")
                        for sub in range(NT // 128):
                            nc.tensor.transpose(
                                ps_tr[:, sub * 128:(sub + 1) * 128],
                                svV[:, sub * 128:(sub + 1) * 128], ident)
                        nc.scalar.copy(
                            vtr[:, m,
                                t * (NT // 128):(t + 1) * (NT // 128), :],
                            ps_tr.rearrange("p (s h) -> p s h", h=128))
                    return emit

                return front, mid, back, vwork

            # ---------------- phase C: attention + out-proj ----------------
            def C_phase(t, m, fillers):
                """Attention for (tile t, head m). fillers: list of (pos,
                closure) inserted after trio index pos. Returns am tile."""
                nch = 4 * t + 4
                qsl = slice(t * NT, (t + 1) * NT)
                ps_attn = psattn.tile([128, NT], F32, tag="pat",
                                      name="ps_attn")
                ps_sum = pssum.tile([1, NT], F32, tag="sum1", name="ps_sum")
                fmap = {}
                for pos, fn in fillers:
                    fmap.setdefault(min(pos, nch - 1), []).append(fn)

                def qk(ci):
                    ps_s = psqk.tile([128, NT], F32, tag="qk", name="ps_s")
                    nc.tensor.matmul(
                        ps_s, kfT[:, m, ci * 128:(ci + 1) * 128],
                        qfT[:, m, qsl], start=True, stop=True)
                    e = expp.tile([128, NT], BF16, tag="e", name="e")
                    nc.scalar.activation(
                        e, ps_s, mybir.ActivationFunctionType.Exp,
                        scale=inv_sqrt_hd)
                    dd = ci * 128 - t * NT
                    if dd >= 0:  # diagonal chunk: causal mask
                        nc.vector.tensor_tensor(
                            e, e, maskb_t[:, 384 - dd:896 - dd],
                            mybir.AluOpType.mult)
                    return e

                epipe = [qk(ci) for ci in range(min(2, nch))]
                for ci in range(nch):
                    if ci + 2 < nch:
                        epipe.append(qk(ci + 2))
                    e = epipe.pop(0)
                    nc.tensor.matmul(ps_attn, vtr[:, m, ci, :], e,
                                     start=(ci == 0), stop=(ci == nch - 1))
                    nc.tensor.matmul(ps_sum, ones_hd, e,
                                     start=(ci == 0), stop=(ci == nch - 1))
                    for fn in fmap.get(ci, ()):
                        fn()
                rrf = scr.tile([1, NT], F32, tag="rrg", bufs=1, name="rrf")
                nc.vector.reciprocal_approx_fast(rrf, ps_sum)
                rr = scr.tile([1, NT], F32R, tag="rr1", bufs=1, name="rr")
                nc.vector.tensor_copy(rr, rrf)
                return ps_attn, rr

            def C_norm_finish(ps_attn, rr):
                ps_rb = psmisc.tile([128, NT], F32, tag="sm", name="ps_rb2")
                nc.tensor.matmul(ps_rb, ones_1, rr, start=True, stop=True)
                rb2 = scr.tile([128, NT], BF16, tag="rb", bufs=4, name="rb2")
                nc.scalar.copy(rb2, ps_rb)
                am = attnp.tile([128, NT], BF16, tag="am", name="am")
                nc.vector.tensor_tensor(am, ps_attn, rb2,
                                        mybir.AluOpType.mult)
                return am

            def outproj(t, ams, fillers):
                qsl = slice(t * NT, (t + 1) * NT)
                fq = list(fillers)
                for i in range(D // 128):
                    ps_o = psqk.tile([128, NT], F32, tag="qk", name="ps_o")
                    for j in range(MPC):
                        nc.tensor.matmul(ps_o, wo_sb[:, j,
                                                     i * 128:(i + 1) * 128],
                                         ams[j], start=(j == 0),
                                         stop=(j == MPC - 1))
                    ost = ostg.tile([128, NT], BF16, tag="ost", name="ost")
                    if i % 4 == 0:
                        nc.scalar.copy(ost, ps_o)
                    else:
                        nc.vector.tensor_copy(ost, ps_o)
                    nc.sync.dma_start(outT[i * 128:(i + 1) * 128, qsl], ost)
                    if i % 4 == 3 and fq:
                        fq.pop(0)()
                while fq:
                    fq.pop(0)()

            # ---------------- emission schedule ----------------
            # A(0); B(0)⋈A(1); then slots: C(t) ⋈ B(t+1) ⋈ A(t+2)
            a_queues = {t: A_units(t) for t in range(NQ)}

            for u in a_queues[0]:
                u()
            emit_x_rest()

            # B(0) woven into A(1)
            a1 = a_queues[1]
            fB, mB, bB, vB = B_units(0)
            fB(0)()
            for u in a1[0:8]:
                u()
            fB(1)()
            for u in a1[8:12]:
                u()
            mB()
            bB(0)()
            bB(1)()
            for u in a1[12:20]:
                u()
            vB(0)()
            vB(1)()
            for u in a1[20:24]:
                u()

            for t in range(NQ):
                has_b = t + 1 < NQ
                has_a = t + 2 < NQ
                aq = list(a_queues[t + 2]) if has_a else []
                ai = [0]

                def take_a(n=1):
                    out = []
                    for _ in range(n):
                        if ai[0] < len(aq):
                            out.append(aq[ai[0]])
                            ai[0] += 1
                    return out

                def a_fill():
                    for u in take_a(1):
                        u()

                if has_b:
                    Bf, Bm, Bb, Bv = B_units(t + 1)

                nch = 4 * t + 4
                # ---- head 0: front(m1) early (DVE), rms mid late ----
                if has_b:
                    Bf(0)()
                fillers = []
                if has_b:
                    fillers.append((1, Bf(1)))
                    fillers.append((nch - 1 if t == 0 else nch - 2, Bm))
                for pos in range(3, nch, 2):
                    fillers.append((pos, a_fill))
                ps_attn0, rr0 = C_phase(t, 0, fillers)
                if has_b:
                    Bb(0)()
                for u in take_a(2):
                    u()
                am0 = C_norm_finish(ps_attn0, rr0)
                # ---- head 1: backs + v-work woven in ----
                fillers = []
                if has_b:
                    fillers.append((0, Bb(1)))
                    fillers.append((2, Bv(0)))
                    fillers.append((nch - 2, Bv(1)))
                for pos in range(3, nch, 2):
                    fillers.append((pos, a_fill))
                ps_attn1, rr1 = C_phase(t, 1, fillers)
                for u in take_a(2):
                    u()
                am1 = C_norm_finish(ps_attn1, rr1)
                # ---- output projection ----
                outproj(t, [am0, am1], [a_fill] * 4)
                while ai[0] < len(aq):
                    aq[ai[0]]()
                    ai[0] += 1

    nc.compile()
    return nc


def _prep_inputs(hidden_states, cos, sin, Wq, Wk, Wv, Wo,
                 conv_q_w, conv_k_w, conv_v_w, q_norm_w, k_norm_w):
    f = np.float32
    bf = ml_dtypes.bfloat16
    x = np.asarray(hidden_states, f)[0]            # [T, D]
    xT = np.ascontiguousarray(x.T.astype(bf))      # [D, T] bf16
    WqT = np.asarray(Wq, f).T                      # [D, D] (col = out ch)
    WkT = np.asarray(Wk, f).T
    WvT = np.asarray(Wv, f).T
    WoT = np.asarray(Wo, f).T                      # [D(in=attn ch), D(out)]

    cosT = np.asarray(cos, f)[0].T                 # [RD, T]
    sinT = np.asarray(sin, f)[0].T

    # trig tables in permuted+stacked layout: part p of the A tile holds
    # channel PI_ROT[p % 64]; ss2 = sign-folded sin for the shuffled partner
    trig = np.zeros((128, 2, T), f)
    for half in range(2):
        for p in range(64):
            c = PI_ROT[p]
            trig[half * 64 + p, 0] = cosT[c]
            trig[half * 64 + p, 1] = (-1.0 if c < 32 else 1.0) * sinT[c]
    trig = trig.astype(bf)

    kl = np.arange(128, dtype=f)[:, None]
    cc = np.arange(896, dtype=f)[None, :]
    maskb = (kl <= cc - 384).astype(bf)

    cw = {0: np.asarray(conv_q_w, f), 1: np.asarray(conv_k_w, f),
          2: np.asarray(conv_v_w, f)}
    qnw = np.asarray(q_norm_w, f)
    knw = np.asarray(k_norm_w, f)

    in_maps = []
    for core in range(NCORES):
        wab = np.zeros((D, NBLK * 128), f)
        convw = np.zeros((128, NBLK, KCONV), f)
        normw = np.zeros((128, 4), f)
        for m in range(MPC):
            h = 2 * core + m
            qch = h * 128 + PI_ROT            # rot q channels (global)
            kch = h * 128 + PI_ROT            # rot k channels
            # block A_m: [q-rot(64) | k-rot(64)], permuted
            bA = 3 * m
            wab[:, bA * 128:bA * 128 + 64] = WqT[:, qch]
            wab[:, bA * 128 + 64:bA * 128 + 128] = WkT[:, kch]
            convw[0:64, bA] = cw[0][qch]
            convw[64:128, bA] = cw[1][kch]
            normw[0:64, 2 * m] = qnw[PI_ROT]
            normw[64:128, 2 * m] = knw[PI_ROT]
            # block B_m: [q-pass(64) | k-pass(64)], natural order
            bB = 3 * m + 1
            pch = h * 128 + 64 + np.arange(64)
            wab[:, bB * 128:bB * 128 + 64] = WqT[:, pch]
            wab[:, bB * 128 + 64:bB * 128 + 128] = WkT[:, pch]
            convw[0:64, bB] = cw[0][pch]
            convw[64:128, bB] = cw[1][pch]
            normw[0:64, 2 * m + 1] = qnw[64:128]
            normw[64:128, 2 * m + 1] = knw[64:128]
            # block V_m: natural
            bV = 3 * m + 2
            vch = h * 128 + np.arange(128)
            wab[:, bV * 128:bV * 128 + 128] = WvT[:, vch]
            convw[:, bV] = cw[2][vch]
        # rms selectors: ones4d[:, 4m+i] picks q/k halves into rstd rows
        ones4d = np.zeros((128, 8), f)
        for m in range(2):
            ones4d[0:64, 4 * m + 2 * m] = 1.0
            ones4d[64:128, 4 * m + 2 * m + 1] = 1.0
        b4 = np.zeros((2, 4, 128), f)
        for m in range(2):
            b4[m, 2 * m, 0:64] = 1.0
            b4[m, 2 * m + 1, 64:128] = 1.0
        sl = slice(core * CPC, (core + 1) * CPC)
        in_maps.append({
            "xT": xT,
            "wab": np.ascontiguousarray(wab.astype(bf)),
            "wo": np.ascontiguousarray(WoT[sl, :].astype(bf)),
            "trig": trig,
            "convw": np.ascontiguousarray(0.5 * convw),
            "normw": normw,
            "maskb": maskb,
            "ones4d": ones4d.astype(bf),
            "b4m0": np.ascontiguousarray(b4[0]),
            "b4m1": np.ascontiguousarray(b4[1]),
        })
    return in_maps


def kernel(hidden_states, cos, sin, Wq, Wk, Wv, Wo,
           conv_q_w, conv_k_w, conv_v_w, q_norm_w, k_norm_w,
           _trace=False):
    global _COMPILED
    if _COMPILED is None:
        _COMPILED = _build()
    nc = _COMPILED
    in_maps = _prep_inputs(hidden_states, cos, sin, Wq, Wk, Wv, Wo,
                           conv_q_w, conv_k_w, conv_v_w, q_norm_w, k_norm_w)
    res = bass_utils.run_bass_kernel_spmd(
        nc, in_maps, core_ids=list(range(NCORES)), trace=_trace)
    acc = np.zeros((D, T), np.float64)
    for r in res.results:
        acc += r["outT"].astype(np.float64)
    out = np.ascontiguousarray(acc.T.astype(np.float32))[None]
    if _trace:
        kernel._last_results = res
    return out
